# revision 24
# baseline (speedup 1.0000x reference)
"""AngularMaxPooling Trainium2 kernel.

Problem: inputs [B=8, R=16, N=6144, F=128] f32.
  norms = ||inputs||_2 over F           -> [B, R, N]
  winner = argmax over R                -> [B, N]
  out[b, n, :] = inputs[b, winner[b,n], n, :]   -> [B, N, F]

Strategy: data-parallel over B across 8 NeuronCores (no collectives).
Per core: stream all R*N*F data once (HWDGE loads, 1 MiB each, 2
rotations per DMA); square on ScalarE (ACT), segmented sum-reduce on
VectorE (norm^2 is argmax-equivalent to the norm); first-occurrence
argmax over R via reduce_max + is_ge mask * descending iota weights;
then gather the winning 512 B rows straight from DRAM with an indirect
(SWDGE) DMA and store contiguously (also SWDGE).

TRN2 walrus codegen allows only ONE sync wait per (non-drain)
instruction, so the whole kernel is engineered so no instruction ever
needs more than one:
  * loads: 8 per group on the HWDGE rotation with xt bufs=8, so a
    slot's previous writer sits on the SAME DMAHW lane (same-proc WAW
    is free); the only remaining wait is the xt WAR vs the square 8
    iterations back.
  * stores + gathers go via SWDGE (Pool) so they never perturb the
    HWDGE lane rotation.
  * squares: tiny ACT "wait carriers" absorb the sq-slot recycling
    waits: cD(k) reads the k-4 reduce's output (carries the DVE WAR),
    and R'(k) right after square(k) reads sq(k), so its own-ACT wait
    value always covers the slot-WAW requirement of square(k+4)
    (engines never implicitly observe their own completion ticks and
    Tile's wait emission is not transitive across engines).
  * everything else (scores, argmax temporaries, gather dests) gets
    one buffer per use -> no recycling hazards at all.
"""

import sys

if "/opt/trn_rl_repo" not in sys.path:
    sys.path.insert(0, "/opt/trn_rl_repo")

import numpy as np

import concourse.bass as bass
import concourse.mybir as mybir
from concourse.bass import IndirectOffsetOnAxis
from concourse.bass_utils import run_bass_kernel_spmd
from concourse.tile import TileContext, add_dep_helper

# Problem shape (hardcoded per spec).
B, R, N, F = 8, 16, 6144, 128
P = 128              # SBUF partitions
J = 8                # vertices per partition per group
V = P * J            # vertices per group (1024)
G = N // V           # groups per core (6)
RB = 2               # rotations per load/square/reduce block
NT = R // RB         # blocks per group (8)
SQB = 4              # sq pool bufs (carrier lookback distance)

FP32 = mybir.dt.float32
U32 = mybir.dt.uint32
COPY = mybir.ActivationFunctionType.Copy
SQUARE = mybir.ActivationFunctionType.Square


def _build() -> bass.Bass:
    nc = bass.Bass()

    x = nc.dram_tensor("x", [R, N, F], FP32, kind="ExternalInput")
    out = nc.dram_tensor("out", [N, F], FP32, kind="ExternalOutput")

    # Flat row view of the input: row (r*N + n) is one vertex's F vector.
    x_rows = x[:, :, :].rearrange("r n f -> (r n) f")

    with TileContext(nc) as tc:
        with (
            tc.tile_pool(name="xt", bufs=NT) as xt_pool,
            tc.tile_pool(name="sq", bufs=SQB) as sq_pool,
            tc.tile_pool(name="sc", bufs=G) as sc_pool,
            tc.tile_pool(name="sm", bufs=G) as sm_pool,
            tc.tile_pool(name="gd", bufs=G * J) as gd_pool,
            tc.tile_pool(name="dp", bufs=4 * G * NT + 16) as dp_pool,
            tc.tile_pool(name="cn", bufs=1) as cn_pool,
        ):
            # Constants.
            # wdesc[p, j*R + r] = R-1 - r  (descending weight over r; the
            # first-occurrence argmax winner maximizes this among ties).
            wdesc = cn_pool.tile([P, J * R], FP32)
            nc.gpsimd.iota(
                wdesc[:, :],
                pattern=[[0, J], [-1, R]],
                base=R - 1,
                channel_multiplier=0,
                allow_small_or_imprecise_dtypes=True,
            )
            # nrel[p, j] = p*J + j  (vertex index within the group).
            nrel = cn_pool.tile([P, J], FP32)
            nc.gpsimd.iota(
                nrel[:, :],
                pattern=[[1, J]],
                base=0,
                channel_multiplier=J,
                allow_small_or_imprecise_dtypes=True,
            )
            # ACT warm-up (absorbs the activation bias-const init wait) and
            # DVE warm-up (absorbs the Pool iota ticks for wdesc/nrel).
            warm = cn_pool.tile([P, 1], FP32)
            nc.gpsimd.memset(warm[:, :], 0.0)
            wa = dp_pool.tile([1, 1], FP32, tag="d")
            nc.scalar.activation(wa[:, :], warm[:1, :1], SQUARE)
            wv = dp_pool.tile([1, 2], FP32, tag="d")
            nc.vector.tensor_tensor(
                out=wv[:, :],
                in0=wdesc[:1, :2],
                in1=nrel[:1, :2],
                op=mybir.AluOpType.add,
            )

            red_hist = {}   # k -> (scores tile, free offset of its slice)
            rprev = None    # previous R' instruction (ordering anchor)
            pend = []       # [(dest tile, j, n0)] awaiting store (1-group delay)

            def flush_stores():
                # Stores on the ACT HWDGE queue; each waits only on its
                # gather's DMASW lane (the DMAHW lane-recycle requirement is
                # already observed via the squares' load waits). Delayed one
                # group so the waits are satisfied on arrival.
                nonlocal pend
                for dst, j, pn0 in pend:
                    # ACT carrier: absorbs the gather's DMASW wait so the
                    # store itself keeps <=1 sync wait (it may still carry a
                    # DMAHW lane-recycle wait, e.g. for the final group).
                    ds = dp_pool.tile([1, 1], FP32, tag="d")
                    nc.scalar.activation(ds[:, :], dst[:1, :1], COPY)
                    nc.scalar.dma_start(
                        out=bass.AP(
                            out[:, :].tensor,
                            (pn0 + j) * F,
                            [[J * F, P], [1, F]],
                        ),
                        in_=dst[:, :],
                    )
                pend = []

            for g in range(G):
                n0 = g * V
                # scores[p, r*J + j] = ||x[r, n0 + p*J + j, :]||^2
                scores = sc_pool.tile([P, R * J], FP32)
                for t in range(NT):
                    k = g * NT + t
                    r0 = t * RB
                    xt = xt_pool.tile([P, RB * J * F], FP32)
                    # DRAM view [p, r, j, f]: p over vertex blocks (step
                    # J*F), r over rotations (step N*F), j in block, f inner.
                    # Issued on the ACT HWDGE queue: program order with the
                    # squares makes both the xt WAR and the slot WAW free.
                    nc.scalar.dma_start(
                        out=xt[:, :],
                        in_=bass.AP(
                            x[:, :, :].tensor,
                            (r0 * N + n0) * F,
                            [[J * F, P], [N * F, RB], [F, J], [1, F]],
                        ),
                    )
                    sq = sq_pool.tile([P, RB * J * F], FP32)
                    cd_i = None
                    if k - SQB in red_hist:
                        # cD: carry the DVE WAR wait for sq slot recycling.
                        st, off = red_hist[k - SQB]
                        dD = dp_pool.tile([1, 1], FP32, tag="d")
                        cd_i = nc.scalar.activation(
                            dD[:, :], st[:1, off : off + 1], COPY
                        )
                    sq_i = nc.scalar.activation(sq[:, :], xt[:, :], SQUARE)
                    if cd_i is not None:
                        add_dep_helper(
                            sq_i.ins, cd_i.ins, sync=False,
                            reason="square after its cD wait carrier",
                        )
                    if rprev is not None:
                        add_dep_helper(
                            sq_i.ins, rprev.ins, sync=False,
                            reason="square after previous R' carrier",
                        )
                    # R': own-tick carrier for the next slot-WAW (see doc).
                    dR = dp_pool.tile([1, 1], FP32, tag="d")
                    rprev = nc.scalar.activation(dR[:, :], sq[:1, :1], COPY)

                    nc.vector.reduce_sum(
                        out=scores[:, r0 * J : (r0 + RB) * J],
                        in_=sq[:, :].rearrange("p (s f) -> p s f", f=F),
                        axis=mybir.AxisListType.X,
                    )
                    red_hist[k] = (scores, r0 * J)

                scores_v = scores[:, :].rearrange("p (r j) -> p j r", r=R)

                # m[p, j] = max over r
                m = sm_pool.tile([P, J], FP32, tag="m")
                nc.vector.reduce_max(
                    out=m[:, :], in_=scores_v, axis=mybir.AxisListType.X
                )
                # mask of score == max (>= max), weighted by R-1-r, maxed:
                # s = R-1 - argmax_first
                geq = sm_pool.tile([P, J * R], FP32, tag="geq")
                nc.vector.tensor_tensor(
                    out=geq[:, :].rearrange("p (j r) -> p j r", r=R),
                    in0=scores_v,
                    in1=m[:, :].broadcast_to((P, J, R)),
                    op=mybir.AluOpType.is_ge,
                )
                sel = sm_pool.tile([P, J * R], FP32, tag="sel")
                nc.vector.tensor_tensor(
                    out=sel[:, :],
                    in0=geq[:, :],
                    in1=wdesc[:, :],
                    op=mybir.AluOpType.mult,
                )
                s = sm_pool.tile([P, J], FP32, tag="s")
                nc.vector.reduce_max(
                    out=s[:, :],
                    in_=sel[:, :].rearrange("p (j r) -> p j r", r=R),
                    axis=mybir.AxisListType.X,
                )
                # winner rotation r* = R-1 - s; gather row = r* * N + n
                #   = (R-1-s)*N + n0 + nrel = -N*s + ((R-1)*N + n0) + nrel
                rowf = sm_pool.tile([P, J], FP32, tag="rowf")
                nc.vector.tensor_scalar(
                    out=rowf[:, :],
                    in0=s[:, :],
                    scalar1=-float(N),
                    scalar2=float((R - 1) * N + n0),
                    op0=mybir.AluOpType.mult,
                    op1=mybir.AluOpType.add,
                )
                rowf2 = sm_pool.tile([P, J], FP32, tag="rowf2")
                nc.vector.tensor_tensor(
                    out=rowf2[:, :],
                    in0=rowf[:, :],
                    in1=nrel[:, :],
                    op=mybir.AluOpType.add,
                )
                rowu = sm_pool.tile([P, J], U32, tag="rowu")
                nc.vector.tensor_copy(out=rowu[:, :], in_=rowf2[:, :])

                # Pool carrier: absorbs the DVE wait (rowu ready) for all of
                # this group's gathers so each gather keeps <=1 sync wait.
                dc = dp_pool.tile([1, 1], FP32, tag="d")
                nc.gpsimd.tensor_copy(out=dc[:, :], in_=rowu[:1, :1])
                # Stores for the previous group go out before this group's
                # gather burst occupies the Pool queue.
                flush_stores()
                # Gather the winning rows (512 B each) from DRAM (SWDGE),
                # one indirect DMA per j-slot: offsets [P, 1] per call (the
                # only offset layout the HW walks correctly).
                for j in range(J):
                    dest = gd_pool.tile([P, F], FP32)
                    nc.gpsimd.indirect_dma_start(
                        out=dest[:, :],
                        out_offset=None,
                        in_=x_rows,
                        in_offset=IndirectOffsetOnAxis(
                            ap=rowu[:, j : j + 1], axis=0
                        ),
                    )
                    pend.append((dest, j, n0))

            flush_stores()

    _split_multiwait_drains(nc)
    return nc


def _split_multiwait_drains(nc: bass.Bass, cap: int = 1) -> None:
    """Walrus codegen rejects instructions with more than one sync wait;
    Tile's kernel-tail drain aggregates one wait per live proc (19 here).
    Split any such drain into a chain of single-wait drains in place."""
    n = 0
    for fn in nc.m.functions:
        for bb in fn.blocks:
            il = bb.instructions  # live list
            i = 0
            while i < len(il):
                ins = il[i]
                si = ins.sync_info
                waits = list(si.on_wait) if si and si.on_wait else []
                if isinstance(ins, mybir.InstDrain) and len(waits) > cap:
                    head, keep = waits[:-cap], waits[-cap:]
                    for w in head:
                        d = mybir.InstDrain(
                            name=f"{ins.name}_wsplit{n}", ins=[], outs=[]
                        )
                        n += 1
                        d.engine = ins.engine
                        d.sync_info = mybir.SyncInfo(on_wait=[w], on_update=[])
                        il.insert(i, d)
                        i += 1
                    ins.sync_info = mybir.SyncInfo(
                        on_wait=keep,
                        on_update=list(si.on_update) if si.on_update else [],
                    )
                i += 1


_NC = None


def _get_nc() -> bass.Bass:
    global _NC
    if _NC is None:
        _NC = _build()
    return _NC


def _run(x: np.ndarray, trace: bool = False):
    assert x.shape == (B, R, N, F), x.shape
    x = np.ascontiguousarray(x, dtype=np.float32)
    nc = _get_nc()
    in_maps = [{"x": x[b]} for b in range(B)]
    res = run_bass_kernel_spmd(nc, in_maps, core_ids=list(range(B)), trace=trace)
    out = np.stack([r["out"] for r in res.results], axis=0)
    return out, res


def kernel(**inputs) -> np.ndarray:
    out, _ = _run(np.asarray(inputs["inputs"]))
    return out


if __name__ == "__main__":
    rng = np.random.default_rng(0)
    x = rng.standard_normal((B, R, N, F), dtype=np.float32)
    out, res = _run(x, trace=False)
    norms = np.linalg.norm(x, axis=-1)
    win = np.argmax(norms, axis=1)
    exp = np.take_along_axis(x, win[:, None, :, None], axis=1)[:, 0]
    err = np.linalg.norm(out - exp) / np.linalg.norm(exp)
    print("Relative error:", err)
